# revision 45
# baseline (speedup 1.0000x reference)
"""Trainium2 Bass kernel for nn_LogReg_455266533602 (histogram_binning).

Math: out[b] = sum_t W[0, text[t, b]] + bias -- the [B,V] histogram times W
collapses to a gather-and-reduce; the histogram is never materialized.

Two balanced paths, data-parallel over batch across 8 NeuronCores.

Measured engine law (TRN2): ap_gather costs ~26.5ns per index per Q7 core
regardless of table size or d (per-idx RD_CMD latency bound), i.e.
~3.3ns/token aggregate across the 8 parallel Q7 cores. The gather path is
therefore Pool-bound and the bulk of the tokens flow through a PE one-hot
path; the split is tuned so Pool and DVE finish together.

  G-path (NA=56 phrases per 128-block): segmented-table ap_gather (16
    segments x 2048 per Q7 core = full vocab per core, host-routed
    offsets), int8 one-hot segment mask, mult+reduce on DVE. All gathers
    are issued up front so the Pool engine grinds from t=0; each select is
    fenced behind a late B-chunk via a zero-scalar data dependency because
    the scheduler's gather cost model is ~15x optimistic and would
    otherwise hoist the selects to the head of the strict-FIFO DVE queue,
    stalling it behind the real 42us-per-1600-idx gathers.
  B-path (72 phrases per block): radix-256 one-hot matmul. Host sends the
    one-hot planes directly as bf16 (no upcast, no is_equal on device),
    merged into one DMA per chunk, issued via the otherwise-idle Act
    engine's DGE so the Sync engine is not serialized:
    G[c, n] = W2dT_lo . ohl + W2dT_hi . ohh   (bf16 matmuls, PSUM f32)
    then one int8 hi-mask multiply + per-phrase X-reduce on DVE, and a
    final ones-matmul folds the 125 c-rows.
"""
import numpy as np

import concourse.bacc as bacc
import concourse.mybir as mybir
import concourse.tile as tile
from concourse.bass_utils import run_bass_kernel_spmd

P = 128
NCORES = 8
SEQ = 100
BPC = 1024              # phrases per NeuronCore
NQ7 = 8                 # Q7 cores per NeuronCore
BLK = BPC // NQ7        # phrases per Q7 core block = 128
NSEG = 16
SEG = 2048
V = 32000

# ---- G-path (gather) ----
NA = 56                 # G phrases per block
NI = NA * SEQ           # idx per Q7 core = 5600
G_CHUNKS = [1600, 1600, 1600, 800]
assert sum(G_CHUNKS) == NI

# ---- B-path (one-hot matmul) ----
NB_PHR = (BLK - NA) * NQ7            # B phrases per core = 576
NB = NB_PHR * SEQ                    # B tokens per core = 57600
BCH = 1600                           # B tokens per chunk (16 phrases)
NBCH = NB // BCH                     # 36 chunks
BPH = BCH // SEQ                     # phrases per B chunk = 16
RAD = 256
NC_HI = 125                          # v >> 8 in [0, 125)

F32 = mybir.dt.float32
BF16 = mybir.dt.bfloat16
I16 = mybir.dt.int16
I8 = mybir.dt.int8

_cached = None


def _build():
    nc = bacc.Bacc("TRN2", debug=False)
    d_table = nc.declare_dram_parameter("table", [P, SEG], F32, isOutput=False)
    d_idx = nc.declare_dram_parameter("idx", [P, NI // 16], I16, isOutput=False)
    d_maskg = [
        nc.declare_dram_parameter(f"maskg{c}", [P, ch], I8, isOutput=False)
        for c, ch in enumerate(G_CHUNKS)
    ]
    d_sel = nc.declare_dram_parameter("sel", [P, NQ7], F32, isOutput=False)
    d_bias8 = nc.declare_dram_parameter("bias8", [NQ7, 1], F32, isOutput=False)
    d_w2dt = nc.declare_dram_parameter("w2dt", [P, 2 * NC_HI], BF16, isOutput=False)
    d_ohb = [nc.declare_dram_parameter(f"ohb{c}", [P, 2 * BCH], BF16, isOutput=False)
             for c in range(NBCH)]
    d_mhi = [nc.declare_dram_parameter(f"mhi{c}", [NC_HI, BCH], I8, isOutput=False)
             for c in range(NBCH)]
    d_ones = nc.declare_dram_parameter("ones", [NC_HI, 1], F32, isOutput=False)
    d_bias1 = nc.declare_dram_parameter("bias1", [1, 1], F32, isOutput=False)
    d_outg = nc.declare_dram_parameter("outg", [NQ7, NA], F32, isOutput=True)
    d_outb = nc.declare_dram_parameter("outb", [1, NB_PHR], F32, isOutput=True)

    with tile.TileContext(nc) as tc:
        with (
            tc.tile_pool(name="const", bufs=1) as cpool,
            tc.tile_pool(name="g", bufs=4) as gpool,
            tc.tile_pool(name="p", bufs=3) as ppool,
            tc.tile_pool(name="ohb", bufs=4) as ohpool,
            tc.tile_pool(name="mb", bufs=4) as mbpool,
            tc.tile_pool(name="pb", bufs=3) as pbpool,
            tc.tile_pool(name="psb", bufs=2, space="PSUM") as psb,
        ):
            t_table = cpool.tile([P, SEG], F32)
            t_idx = cpool.tile([P, NI // 16], I16)
            t_maskg = [cpool.tile([P, ch], I8, name=f"t_mg{c}", tag=f"mg{c}")
                       for c, ch in enumerate(G_CHUNKS)]
            t_sel = cpool.tile([P, NQ7], F32)
            t_bias8 = cpool.tile([NQ7, 1], F32)
            t_w2dt = cpool.tile([P, 2 * NC_HI], BF16)
            t_ones = cpool.tile([NC_HI, 1], F32)
            t_bias1 = cpool.tile([1, 1], F32)
            t_psg = cpool.tile([P, NA], F32)
            t_psb = cpool.tile([NC_HI, NB_PHR], F32)

            # only the gather-critical DMAs ride Sync: the gpsimd library
            # IRAM load (triggered by the pool config) otherwise queues
            # behind them and delays the first gather by several us
            nc.sync.dma_start(out=t_table[:], in_=d_table[:])
            nc.sync.dma_start(out=t_idx[:], in_=d_idx[:])
            nc.scalar.dma_start(out=t_w2dt[:], in_=d_w2dt[:])
            nc.scalar.dma_start(out=t_sel[:], in_=d_sel[:])
            nc.scalar.dma_start(out=t_bias8[:], in_=d_bias8[:])
            nc.scalar.dma_start(out=t_ones[:], in_=d_ones[:])
            nc.scalar.dma_start(out=t_bias1[:], in_=d_bias1[:])
            for c, ch in enumerate(G_CHUNKS):
                nc.scalar.dma_start(out=t_maskg[c][:], in_=d_maskg[c][:])

            # ---- all gathers up front ----
            t_gather = []
            goff = 0
            for c, ch in enumerate(G_CHUNKS):
                t_g = gpool.tile([P, max(G_CHUNKS)], F32, name=f"t_g{c}",
                                 tag=f"g{c}")
                nc.gpsimd.ap_gather(
                    out_ap=t_g[:, :ch],
                    in_ap=t_table[:],
                    idxs_ap=t_idx[:, goff // 16:(goff + ch) // 16],
                    channels=P, num_elems=SEG, d=1, num_idxs=ch)
                t_gather.append((c, goff, ch, t_g))
                goff += ch

            def g_select(c, off, ch, t_g, t_fence):
                t_p = ppool.tile([P, max(G_CHUNKS)], F32, tag="p")
                nc.vector.scalar_tensor_tensor(
                    out=t_p[:, :ch], in0=t_g[:, :ch], scalar=t_fence[:],
                    in1=t_maskg[c][:],
                    op0=mybir.AluOpType.add, op1=mybir.AluOpType.mult)
                nc.vector.tensor_reduce(
                    out=t_psg[:, off // SEQ:(off + ch) // SEQ],
                    in_=t_p[:, :ch].rearrange("p (b t) -> p b t", t=SEQ),
                    axis=mybir.AxisListType.X,
                    op=mybir.AluOpType.add)

            def b_chunk(c):
                t_ohb = ohpool.tile([P, 2 * BCH], BF16, tag="ohb")
                b_chunk.last_ohb = t_ohb
                t_mhi = mbpool.tile([NC_HI, BCH], I8, tag="mhi")
                nc.scalar.dma_start(out=t_ohb[:], in_=d_ohb[c][:])
                nc.sync.dma_start(out=t_mhi[:], in_=d_mhi[c][:])
                t_gb = psb.tile([NC_HI, BCH], F32, tag="gb")
                pieces = [(0, 512), (512, 512), (1024, 512), (1536, 64)]
                for q, w in pieces:
                    nc.tensor.matmul(out=t_gb[:, q:q + w],
                                     lhsT=t_w2dt[:, :NC_HI],
                                     rhs=t_ohb[:, q:q + w],
                                     start=True, stop=False)
                for q, w in pieces:
                    nc.tensor.matmul(out=t_gb[:, q:q + w],
                                     lhsT=t_w2dt[:, NC_HI:],
                                     rhs=t_ohb[:, BCH + q:BCH + q + w],
                                     start=False, stop=True)
                t_pb = pbpool.tile([NC_HI, BCH], BF16, tag="pbx")
                nc.vector.tensor_tensor(
                    out=t_pb[:], in0=t_gb[:], in1=t_mhi[:],
                    op=mybir.AluOpType.mult)
                nc.vector.tensor_reduce(
                    out=t_psb[:, c * BPH:(c + 1) * BPH],
                    in_=t_pb[:].rearrange("p (b t) -> p b t", t=SEQ),
                    axis=mybir.AxisListType.X,
                    op=mybir.AluOpType.add)

            def make_fence():
                t_f = cpool.tile([P, 1], F32, name=f"t_f{make_fence.n}",
                                 tag=f"f{make_fence.n}")
                make_fence.n += 1
                nc.vector.tensor_scalar(
                    out=t_f[:], in0=b_chunk.last_ohb[:, 0:1], scalar1=0.0,
                    scalar2=None, op0=mybir.AluOpType.mult)
                return t_f
            make_fence.n = 0

            # fence positions: DVE reaches B-chunk k at roughly 18+4.5k us;
            # gather c really completes at ~55/98/140/161 us
            sel_at = {10: [0], 19: [1], 29: [2], NBCH - 1: [3]}
            for c in range(NBCH):
                b_chunk(c)
                if c in sel_at:
                    t_f = make_fence()
                    for gidx in sel_at[c]:
                        g_select(*t_gather[gidx], t_f)

            # ---- finals ----
            t_accg = psb.tile([NQ7, NA], F32, tag="gb")
            nc.tensor.matmul(out=t_accg[:], lhsT=t_sel[:], rhs=t_psg[:],
                             start=True, stop=True)
            t_outg = cpool.tile([NQ7, NA], F32)
            nc.vector.tensor_scalar(
                out=t_outg[:], in0=t_accg[:], scalar1=t_bias8[:], scalar2=None,
                op0=mybir.AluOpType.add)
            nc.sync.dma_start(out=d_outg[:], in_=t_outg[:])

            t_accb = psb.tile([1, NB_PHR], F32, tag="gb")
            for q in range(0, NB_PHR, 512):
                w = min(512, NB_PHR - q)
                nc.tensor.matmul(out=t_accb[:, q:q + w], lhsT=t_ones[:],
                                 rhs=t_psb[:, q:q + w], start=True, stop=True)
            t_outb = cpool.tile([1, NB_PHR], F32)
            nc.vector.tensor_scalar(
                out=t_outb[:], in0=t_accb[:], scalar1=t_bias1[:], scalar2=None,
                op0=mybir.AluOpType.add)
            nc.sync.dma_start(out=d_outb[:], in_=t_outb[:])
    nc.compile()
    return nc


def _prep_inputs(text: np.ndarray, W: np.ndarray, b: np.ndarray):
    import ml_dtypes
    wpad = np.zeros(NSEG * SEG, np.float32)
    wpad[:V] = W[0].astype(np.float32)
    table = np.tile(wpad.reshape(NSEG, SEG), (NQ7, 1))      # [128, 2048]
    sel = np.repeat(np.eye(NQ7, dtype=np.float32), NSEG, axis=0)  # [128, 8]
    bias8 = np.full((NQ7, 1), np.float32(b[0]), np.float32)
    bias1 = np.full((1, 1), np.float32(b[0]), np.float32)
    ones = np.ones((NC_HI, 1), np.float32)
    # W2dT[k, c] = W[c*256 + k] (lo half) | W[c*256 + 128 + k] (hi half)
    w2d = wpad.reshape(NC_HI + 3, RAD)[:NC_HI]       # [125, 256]
    w2dt = np.concatenate([w2d[:, :P].T, w2d[:, P:].T], axis=1)  # [128, 250]
    w2dt = np.ascontiguousarray(w2dt).astype(ml_dtypes.bfloat16)

    text = np.asarray(text)
    iota128 = np.arange(P, dtype=np.int16)[:, None]
    in_maps = []
    for c in range(NCORES):
        vp = np.ascontiguousarray(text[:, c * BPC:(c + 1) * BPC].T)  # [1024, 100]
        v3 = vp.reshape(NQ7, BLK, SEQ).astype(np.int64)
        # ---- G: first NA phrases of each block ----
        vg = v3[:, :NA, :].reshape(NQ7, NI)
        off = (vg & (SEG - 1)).astype(np.int16)
        seg = (vg >> 11).astype(np.int8)
        idx = off.reshape(NQ7, NI // 16, 16).transpose(0, 2, 1).reshape(P, NI // 16)
        maskg = (seg[:, None, :] == np.arange(NSEG, dtype=np.int8)[None, :, None]
                 ).astype(np.int8).reshape(P, NI)
        # ---- B: remaining phrases, flat (block, phrase, t) order ----
        vb = v3[:, NA:, :].reshape(NB)
        a = (vb & (RAD - 1)).astype(np.int16)
        hi = (vb >> 8).astype(np.int8)
        ohl = (a[None, :] == iota128).astype(ml_dtypes.bfloat16)         # [128, NB]
        ohh = (a[None, :] == (iota128 + P)).astype(ml_dtypes.bfloat16)   # [128, NB]
        mhi = (hi[None, :] == np.arange(NC_HI, dtype=np.int8)[:, None]
               ).astype(np.int8)                                          # [125, NB]
        in_map = {
            "table": table, "idx": np.ascontiguousarray(idx),
            "sel": sel, "bias8": bias8, "bias1": bias1, "ones": ones,
            "w2dt": w2dt,
        }
        for ci, ch in enumerate(G_CHUNKS):
            off_c = sum(G_CHUNKS[:ci])
            in_map[f"maskg{ci}"] = np.ascontiguousarray(maskg[:, off_c:off_c + ch])
        for ci in range(NBCH):
            s = slice(ci * BCH, (ci + 1) * BCH)
            in_map[f"ohb{ci}"] = np.ascontiguousarray(
                np.concatenate([ohl[:, s], ohh[:, s]], axis=1))
            in_map[f"mhi{ci}"] = np.ascontiguousarray(mhi[:, s])
        in_maps.append(in_map)
    return in_maps


def kernel(text: np.ndarray, W: np.ndarray, b: np.ndarray) -> np.ndarray:
    global _cached
    if _cached is None:
        _cached = _build()
    nc = _cached
    in_maps = _prep_inputs(np.asarray(text), np.asarray(W), np.asarray(b))
    res = run_bass_kernel_spmd(nc, in_maps, list(range(NCORES)))
    full = np.empty((NCORES, NQ7, BLK), np.float32)
    for c in range(NCORES):
        og = res.results[c]["outg"].reshape(NQ7, NA)
        ob = res.results[c]["outb"].reshape(NQ7, BLK - NA)
        full[c, :, :NA] = og
        full[c, :, NA:] = ob
    return full.reshape(NCORES * BPC, 1).astype(np.float32)


if __name__ == "__main__":
    rng = np.random.default_rng(0)
    text = rng.integers(0, V, size=(SEQ, BPC * NCORES)).astype(np.int64)
    W = rng.standard_normal((1, V)).astype(np.float32)
    b = np.zeros(1, np.float32)
    got = kernel(text, W, b)
    exp = (W[0][text].sum(axis=0) + b[0]).reshape(-1, 1).astype(np.float32)
    err = np.abs(got - exp).max() / np.abs(exp).max()
    print("max abs rel err:", err)
    print("OK" if err < 5e-3 else "FAIL")


# revision 46
# speedup vs baseline: 1.1530x; 1.1530x over previous
"""Trainium2 Bass kernel for nn_LogReg_455266533602 (histogram_binning).

Math: out[b] = sum_t W[0, text[t, b]] + bias -- the [B,V] histogram times W
collapses to a gather-and-reduce; the histogram is never materialized.

Two balanced paths, data-parallel over batch across 8 NeuronCores.

Measured engine law (TRN2): ap_gather costs ~26.5ns per index per Q7 core
regardless of table size or d (per-idx RD_CMD latency bound), i.e.
~3.3ns/token aggregate across the 8 parallel Q7 cores. The gather path is
therefore Pool-bound and the bulk of the tokens flow through a PE one-hot
path; the split is tuned so Pool and DVE finish together.

  G-path (NA=56 phrases per 128-block): segmented-table ap_gather (16
    segments x 2048 per Q7 core = full vocab per core, host-routed
    offsets), int8 one-hot segment mask, mult+reduce on DVE. All gathers
    are issued up front so the Pool engine grinds from t=0; each select is
    fenced behind a late B-chunk via a zero-scalar data dependency because
    the scheduler's gather cost model is ~15x optimistic and would
    otherwise hoist the selects to the head of the strict-FIFO DVE queue,
    stalling it behind the real 42us-per-1600-idx gathers.
  B-path (72 phrases per block): radix-256 one-hot matmul. Host sends the
    one-hot planes directly as bf16 (no upcast, no is_equal on device),
    merged into one DMA per chunk, issued via the otherwise-idle Act
    engine's DGE so the Sync engine is not serialized:
    G[c, n] = W2dT_lo . ohl + W2dT_hi . ohh   (bf16 matmuls, PSUM f32)
    then one int8 hi-mask multiply + per-phrase X-reduce on DVE, and a
    final ones-matmul folds the 125 c-rows.
"""
import numpy as np

import concourse.bacc as bacc
import concourse.mybir as mybir
import concourse.tile as tile
from concourse.bass_utils import run_bass_kernel_spmd

P = 128
NCORES = 8
SEQ = 100
BPC = 1024              # phrases per NeuronCore
NQ7 = 8                 # Q7 cores per NeuronCore
BLK = BPC // NQ7        # phrases per Q7 core block = 128
NSEG = 16
SEG = 2048
V = 32000

# ---- G-path (gather) ----
NA = 56                 # G phrases per block
NI = NA * SEQ           # idx per Q7 core = 5600
G_CHUNKS = [1600, 1600, 1600, 800]
assert sum(G_CHUNKS) == NI

# ---- B-path (one-hot matmul) ----
NB_PHR = (BLK - NA) * NQ7            # B phrases per core = 576
NB = NB_PHR * SEQ                    # B tokens per core = 57600
BCH = 1600                           # B tokens per chunk (16 phrases)
NBCH = NB // BCH                     # 36 chunks
BPH = BCH // SEQ                     # phrases per B chunk = 16
RAD = 256
NC_HI = 125                          # v >> 8 in [0, 125)

F32 = mybir.dt.float32
BF16 = mybir.dt.bfloat16
I16 = mybir.dt.int16
I8 = mybir.dt.int8

_cached = None


def _build():
    nc = bacc.Bacc("TRN2", debug=False)
    d_table = nc.declare_dram_parameter("table", [P, SEG], F32, isOutput=False)
    d_idx = nc.declare_dram_parameter("idx", [P, NI // 16], I16, isOutput=False)
    d_maskg = [
        nc.declare_dram_parameter(f"maskg{c}", [P, ch], I8, isOutput=False)
        for c, ch in enumerate(G_CHUNKS)
    ]
    d_sel = nc.declare_dram_parameter("sel", [P, NQ7], F32, isOutput=False)
    d_bias8 = nc.declare_dram_parameter("bias8", [NQ7, 1], F32, isOutput=False)
    d_w2dt = nc.declare_dram_parameter("w2dt", [P, 2 * NC_HI], BF16, isOutput=False)
    d_ohb = [nc.declare_dram_parameter(f"ohb{c}", [P, 2 * BCH], BF16, isOutput=False)
             for c in range(NBCH)]
    d_mhi = [nc.declare_dram_parameter(f"mhi{c}", [NC_HI, BCH], I8, isOutput=False)
             for c in range(NBCH)]
    d_ones = nc.declare_dram_parameter("ones", [NC_HI, 1], F32, isOutput=False)
    d_bias1 = nc.declare_dram_parameter("bias1", [1, 1], F32, isOutput=False)
    d_outg = nc.declare_dram_parameter("outg", [NQ7, NA], F32, isOutput=True)
    d_outb = nc.declare_dram_parameter("outb", [1, NB_PHR], F32, isOutput=True)

    with tile.TileContext(nc) as tc:
        with (
            tc.tile_pool(name="const", bufs=1) as cpool,
            tc.tile_pool(name="g", bufs=4) as gpool,
            tc.tile_pool(name="p", bufs=3) as ppool,
            tc.tile_pool(name="ohb", bufs=4) as ohpool,
            tc.tile_pool(name="mb", bufs=4) as mbpool,
            tc.tile_pool(name="pb", bufs=3) as pbpool,
            tc.tile_pool(name="psb", bufs=2, space="PSUM") as psb,
        ):
            t_table = cpool.tile([P, SEG], F32)
            t_idx = cpool.tile([P, NI // 16], I16)
            t_maskg = [cpool.tile([P, ch], I8, name=f"t_mg{c}", tag=f"mg{c}")
                       for c, ch in enumerate(G_CHUNKS)]
            t_sel = cpool.tile([P, NQ7], F32)
            t_bias8 = cpool.tile([NQ7, 1], F32)
            t_w2dt = cpool.tile([P, 2 * NC_HI], BF16)
            t_ones = cpool.tile([NC_HI, 1], F32)
            t_bias1 = cpool.tile([1, 1], F32)
            t_psg = cpool.tile([P, NA], F32)
            t_psb = cpool.tile([NC_HI, NB_PHR], F32)

            # only the gather-critical DMAs ride Sync: the gpsimd library
            # IRAM load (triggered by the pool config) otherwise queues
            # behind them and delays the first gather by several us
            nc.sync.dma_start(out=t_table[:], in_=d_table[:])
            nc.sync.dma_start(out=t_idx[:], in_=d_idx[:])
            nc.scalar.dma_start(out=t_w2dt[:], in_=d_w2dt[:])
            nc.scalar.dma_start(out=t_sel[:], in_=d_sel[:])
            nc.scalar.dma_start(out=t_bias8[:], in_=d_bias8[:])
            nc.scalar.dma_start(out=t_ones[:], in_=d_ones[:])
            nc.scalar.dma_start(out=t_bias1[:], in_=d_bias1[:])
            for c, ch in enumerate(G_CHUNKS):
                nc.scalar.dma_start(out=t_maskg[c][:], in_=d_maskg[c][:])

            # ---- all gathers up front ----
            t_gather = []
            goff = 0
            for c, ch in enumerate(G_CHUNKS):
                t_g = gpool.tile([P, max(G_CHUNKS)], F32, name=f"t_g{c}",
                                 tag=f"g{c}")
                nc.gpsimd.ap_gather(
                    out_ap=t_g[:, :ch],
                    in_ap=t_table[:],
                    idxs_ap=t_idx[:, goff // 16:(goff + ch) // 16],
                    channels=P, num_elems=SEG, d=1, num_idxs=ch)
                t_gather.append((c, goff, ch, t_g))
                goff += ch

            def g_select(c, off, ch, t_g, t_fence):
                t_p = ppool.tile([P, max(G_CHUNKS)], F32, tag="p")
                nc.vector.scalar_tensor_tensor(
                    out=t_p[:, :ch], in0=t_g[:, :ch], scalar=t_fence[:],
                    in1=t_maskg[c][:],
                    op0=mybir.AluOpType.add, op1=mybir.AluOpType.mult)
                nc.vector.tensor_reduce(
                    out=t_psg[:, off // SEQ:(off + ch) // SEQ],
                    in_=t_p[:, :ch].rearrange("p (b t) -> p b t", t=SEQ),
                    axis=mybir.AxisListType.X,
                    op=mybir.AluOpType.add)

            def b_chunk(c):
                t_ohb = ohpool.tile([P, 2 * BCH], BF16, tag="ohb")
                b_chunk.last_ohb = t_ohb
                t_mhi = mbpool.tile([NC_HI, BCH], I8, tag="mhi")
                nc.scalar.dma_start(out=t_ohb[:], in_=d_ohb[c][:])
                nc.scalar.dma_start(out=t_mhi[:], in_=d_mhi[c][:])
                t_gb = psb.tile([NC_HI, BCH], F32, tag="gb")
                pieces = [(0, 512), (512, 512), (1024, 512), (1536, 64)]
                for q, w in pieces:
                    nc.tensor.matmul(out=t_gb[:, q:q + w],
                                     lhsT=t_w2dt[:, :NC_HI],
                                     rhs=t_ohb[:, q:q + w],
                                     start=True, stop=False)
                for q, w in pieces:
                    nc.tensor.matmul(out=t_gb[:, q:q + w],
                                     lhsT=t_w2dt[:, NC_HI:],
                                     rhs=t_ohb[:, BCH + q:BCH + q + w],
                                     start=False, stop=True)
                t_pb = pbpool.tile([NC_HI, BCH], BF16, tag="pbx")
                nc.vector.tensor_tensor(
                    out=t_pb[:], in0=t_gb[:], in1=t_mhi[:],
                    op=mybir.AluOpType.mult)
                nc.vector.tensor_reduce(
                    out=t_psb[:, c * BPH:(c + 1) * BPH],
                    in_=t_pb[:].rearrange("p (b t) -> p b t", t=SEQ),
                    axis=mybir.AxisListType.X,
                    op=mybir.AluOpType.add)

            def make_fence():
                t_f = cpool.tile([P, 1], F32, name=f"t_f{make_fence.n}",
                                 tag=f"f{make_fence.n}")
                make_fence.n += 1
                nc.vector.tensor_scalar(
                    out=t_f[:], in0=b_chunk.last_ohb[:, 0:1], scalar1=0.0,
                    scalar2=None, op0=mybir.AluOpType.mult)
                return t_f
            make_fence.n = 0

            # fence positions: DVE reaches B-chunk k at roughly 18+4.5k us;
            # gather c really completes at ~55/98/140/161 us
            sel_at = {10: [0], 19: [1], 29: [2], NBCH - 1: [3]}
            for c in range(NBCH):
                b_chunk(c)
                if c in sel_at:
                    t_f = make_fence()
                    for gidx in sel_at[c]:
                        g_select(*t_gather[gidx], t_f)

            # ---- finals ----
            t_accg = psb.tile([NQ7, NA], F32, tag="gb")
            nc.tensor.matmul(out=t_accg[:], lhsT=t_sel[:], rhs=t_psg[:],
                             start=True, stop=True)
            t_outg = cpool.tile([NQ7, NA], F32)
            nc.vector.tensor_scalar(
                out=t_outg[:], in0=t_accg[:], scalar1=t_bias8[:], scalar2=None,
                op0=mybir.AluOpType.add)
            nc.sync.dma_start(out=d_outg[:], in_=t_outg[:])

            t_accb = psb.tile([1, NB_PHR], F32, tag="gb")
            for q in range(0, NB_PHR, 512):
                w = min(512, NB_PHR - q)
                nc.tensor.matmul(out=t_accb[:, q:q + w], lhsT=t_ones[:],
                                 rhs=t_psb[:, q:q + w], start=True, stop=True)
            t_outb = cpool.tile([1, NB_PHR], F32)
            nc.vector.tensor_scalar(
                out=t_outb[:], in0=t_accb[:], scalar1=t_bias1[:], scalar2=None,
                op0=mybir.AluOpType.add)
            nc.sync.dma_start(out=d_outb[:], in_=t_outb[:])
    nc.compile()
    return nc


def _prep_inputs(text: np.ndarray, W: np.ndarray, b: np.ndarray):
    import ml_dtypes
    wpad = np.zeros(NSEG * SEG, np.float32)
    wpad[:V] = W[0].astype(np.float32)
    table = np.tile(wpad.reshape(NSEG, SEG), (NQ7, 1))      # [128, 2048]
    sel = np.repeat(np.eye(NQ7, dtype=np.float32), NSEG, axis=0)  # [128, 8]
    bias8 = np.full((NQ7, 1), np.float32(b[0]), np.float32)
    bias1 = np.full((1, 1), np.float32(b[0]), np.float32)
    ones = np.ones((NC_HI, 1), np.float32)
    # W2dT[k, c] = W[c*256 + k] (lo half) | W[c*256 + 128 + k] (hi half)
    w2d = wpad.reshape(NC_HI + 3, RAD)[:NC_HI]       # [125, 256]
    w2dt = np.concatenate([w2d[:, :P].T, w2d[:, P:].T], axis=1)  # [128, 250]
    w2dt = np.ascontiguousarray(w2dt).astype(ml_dtypes.bfloat16)

    text = np.asarray(text)
    iota128 = np.arange(P, dtype=np.int16)[:, None]
    in_maps = []
    for c in range(NCORES):
        vp = np.ascontiguousarray(text[:, c * BPC:(c + 1) * BPC].T)  # [1024, 100]
        v3 = vp.reshape(NQ7, BLK, SEQ).astype(np.int64)
        # ---- G: first NA phrases of each block ----
        vg = v3[:, :NA, :].reshape(NQ7, NI)
        off = (vg & (SEG - 1)).astype(np.int16)
        seg = (vg >> 11).astype(np.int8)
        idx = off.reshape(NQ7, NI // 16, 16).transpose(0, 2, 1).reshape(P, NI // 16)
        maskg = (seg[:, None, :] == np.arange(NSEG, dtype=np.int8)[None, :, None]
                 ).astype(np.int8).reshape(P, NI)
        # ---- B: remaining phrases, flat (block, phrase, t) order ----
        vb = v3[:, NA:, :].reshape(NB)
        a = (vb & (RAD - 1)).astype(np.int16)
        hi = (vb >> 8).astype(np.int8)
        ohl = (a[None, :] == iota128).astype(ml_dtypes.bfloat16)         # [128, NB]
        ohh = (a[None, :] == (iota128 + P)).astype(ml_dtypes.bfloat16)   # [128, NB]
        mhi = (hi[None, :] == np.arange(NC_HI, dtype=np.int8)[:, None]
               ).astype(np.int8)                                          # [125, NB]
        in_map = {
            "table": table, "idx": np.ascontiguousarray(idx),
            "sel": sel, "bias8": bias8, "bias1": bias1, "ones": ones,
            "w2dt": w2dt,
        }
        for ci, ch in enumerate(G_CHUNKS):
            off_c = sum(G_CHUNKS[:ci])
            in_map[f"maskg{ci}"] = np.ascontiguousarray(maskg[:, off_c:off_c + ch])
        for ci in range(NBCH):
            s = slice(ci * BCH, (ci + 1) * BCH)
            in_map[f"ohb{ci}"] = np.ascontiguousarray(
                np.concatenate([ohl[:, s], ohh[:, s]], axis=1))
            in_map[f"mhi{ci}"] = np.ascontiguousarray(mhi[:, s])
        in_maps.append(in_map)
    return in_maps


def kernel(text: np.ndarray, W: np.ndarray, b: np.ndarray) -> np.ndarray:
    global _cached
    if _cached is None:
        _cached = _build()
    nc = _cached
    in_maps = _prep_inputs(np.asarray(text), np.asarray(W), np.asarray(b))
    res = run_bass_kernel_spmd(nc, in_maps, list(range(NCORES)))
    full = np.empty((NCORES, NQ7, BLK), np.float32)
    for c in range(NCORES):
        og = res.results[c]["outg"].reshape(NQ7, NA)
        ob = res.results[c]["outb"].reshape(NQ7, BLK - NA)
        full[c, :, :NA] = og
        full[c, :, NA:] = ob
    return full.reshape(NCORES * BPC, 1).astype(np.float32)


if __name__ == "__main__":
    rng = np.random.default_rng(0)
    text = rng.integers(0, V, size=(SEQ, BPC * NCORES)).astype(np.int64)
    W = rng.standard_normal((1, V)).astype(np.float32)
    b = np.zeros(1, np.float32)
    got = kernel(text, W, b)
    exp = (W[0][text].sum(axis=0) + b[0]).reshape(-1, 1).astype(np.float32)
    err = np.abs(got - exp).max() / np.abs(exp).max()
    print("max abs rel err:", err)
    print("OK" if err < 5e-3 else "FAIL")
